# revision 2
# baseline (speedup 1.0000x reference)
"""GAT layer (N=8192, D=64) as a Bass/Tile kernel on 8 TRN2 NeuronCores.

Math (reference):
    h  = x @ W.T + b
    s1 = h @ a1 ; s2 = h @ a2                    # [N] each
    score[i,j] = s2[i] + s1[j]
    att = softmax_j(leaky_relu(score))
    out = att @ x

Reformulation used here:
    Fold the linear layer:  v = W.T @ [a1|a2], c_k = b.a_k
      p1 = x @ v1 ; p2 = x @ v2 ; s1 = p1 + c1 ; s2 = p2 + c2
    Softmax rows are shift invariant, so subtract p2[i] from row i:
      e[j,i] = exp(lr(score[i,j]) - p2[i])
             = max( E1[j], F1[j] * G2[i] )
      with E1 = exp(sh1), F1 = exp(0.01*sh1), sh1[j] = p1[j]+c1+c2,
      G2[i] = exp(-0.99*p2[i])  (exp is monotone so exp(max)=max(exp)).
    With per-j-row scalars E1[j], F1[j] and a broadcast tile
    G2b[j,i] = G2[i], the unnormalized weight tile (layout
    [j partitions, i free]) is ONE DVE tensor_scalar op:
      e[j,i] = max( G2b[j,i] * F1[j],  E1[j] )
    The final matmul (with a ones-column appended to x to get the
    softmax denominator for free) accumulates over j in PSUM:
      outT[0:64, i] += x_ext[j,:].T @ e[j, i] ; Z[i] = outT[64, i]

Sharding: each core owns N/8 = 1024 query rows i; x is replicated.
Per-core layout trick: x columns (j tiles) are rotated on the host so
every core sees its OWN i-block as j-tiles 0..7 — the kernel is fully
SPMD and the own-block slice (needed for G2b) arrives first.

Engine roles: PE computes v/c, the s1 projections (64 tiny matmuls off
a d-major xT copy), G2b broadcast, the 128 accumulating main matmuls
and the epilogue transposes.  ACT computes all exps (E1/F1/G2 rows)
and PSUM->SBUF copies.  DVE runs the 64 fused mult+max tensor_scalar
e-tiles (the critical stream, ~456ns each) plus the tiny epilogue
normalize.  DMA is chunked and interleaved by need-time.
"""

import sys
import types

import ml_dtypes
import numpy as np

import concourse.bacc as bacc
import concourse.bass as bass
import concourse.mybir as mybir
import concourse.tile as tile
from concourse.bass_utils import run_bass_kernel_spmd


def _install_ntff_hook_shim():
    """The agent image's ``antenv`` lacks ``axon_hooks``; provide it so
    ``run_bass_kernel_spmd(trace=True)`` can capture NTFF profiles."""
    if "antenv.axon_hooks" in sys.modules:
        return
    try:
        from trn_agent_boot.trn_boot import _ntff_profile_via_ctypes

        hook = _ntff_profile_via_ctypes("/opt/axon/libaxon_pjrt.so")
        mod = types.ModuleType("antenv.axon_hooks")
        mod._hook = hook
        mod.get_axon_ntff_profile_hook = lambda: mod._hook
        mod.set_axon_ntff_profile_hook = lambda h: setattr(mod, "_hook", h)
        sys.modules["antenv.axon_hooks"] = mod
    except Exception:
        pass


_install_ntff_hook_shim()

N, D = 8192, 64
NCORES = 8
RB = N // NCORES          # rows (i) per core = 1024
NT = N // 128             # j tiles of 128 = 64
BT = RB // 128            # i tiles per core = 8
F32 = mybir.dt.float32
BF16 = mybir.dt.bfloat16
EXP = mybir.ActivationFunctionType.Exp
ADD = mybir.AluOpType.add
MUL = mybir.AluOpType.mult
MAX = mybir.AluOpType.max
AX_X = mybir.AxisListType.X
PKW = D + 131  # packed small-input width (W | b | a | ident)


def build_bass() -> bass.Bass:
    nc = bacc.Bacc(None)
    # xT: d-major transposed x [64, N] f32, columns rotated so the core's
    # own i-block is cols 0..RB-1.  xbf: partition-major (p, t, 128) bf16
    # x with a ones column at col 64, tiles rotated the same way.
    xT_d = nc.declare_dram_parameter("xT", [D, N], F32, isOutput=False)
    xbf_d = nc.declare_dram_parameter(
        "xbf", [128, NT * 128], BF16, isOutput=False
    )
    pk_d = nc.declare_dram_parameter("pack", [128, PKW], F32, isOutput=False)
    out_d = nc.declare_dram_parameter("out", [128, BT * D], F32, isOutput=True)

    with tile.TileContext(nc) as tc:
        with (
            tc.tile_pool(name="persist", bufs=1) as persist,
            tc.tile_pool(name="small", bufs=1) as small,
            tc.tile_pool(name="epool", bufs=12) as epool,
            tc.tile_pool(name="opool", bufs=2) as opool,
            tc.tile_pool(name="psumA", bufs=3, space="PSUM") as psumA,
            tc.tile_pool(name="psumB", bufs=1, space="PSUM") as psumB,
            tc.tile_pool(name="psumP", bufs=2, space="PSUM") as psumP,
        ):
            # ------- all small inputs arrive in ONE packed DMA -------
            pk = small.tile([128, PKW], F32)
            nc.sync.dma_start(pk, pk_d[:, :])
            W_sb = pk[0:D, 0:D]
            b_sb = pk[0:D, D : D + 1]
            a_sb = pk[0:D, D + 1 : D + 3]
            ident = pk[:, D + 3 : D + 3 + 128]
            ones_row = small.tile([1, 128], F32)
            nc.vector.memset(ones_row, 1.0)
            ones_bf = small.tile([1, 128], BF16)
            nc.vector.memset(ones_bf, 1.0)

            # ------- x loads, interleaved by first-use time -------
            xT_sb = persist.tile([D, N], F32)
            nc.sync.dma_start(xT_sb[:, 0:RB], xT_d[:, 0:RB])  # own block
            xbf_flat = persist.tile([128, NT * 128], BF16)
            x_bf = xbf_flat.rearrange("p (t d) -> p t d", t=NT)
            CWB = 16 * 128  # 16 j-tiles of bf16 x per chunk
            xt_chunks = [(RB, 3072), (3072, 5632), (5632, N)]
            nc.sync.dma_start(
                xbf_flat[:, 0 * CWB : 1 * CWB], xbf_d[:, 0 * CWB : 1 * CWB]
            )
            for ci, (lo, hi) in enumerate(xt_chunks):
                nc.sync.dma_start(xT_sb[:, lo:hi], xT_d[:, lo:hi])
                if ci + 1 < 4:
                    nc.sync.dma_start(
                        xbf_flat[:, (ci + 1) * CWB : (ci + 2) * CWB],
                        xbf_d[:, (ci + 1) * CWB : (ci + 2) * CWB],
                    )

            # ---------------- tiny projections on PE ----------------
            # v = W.T @ [a1|a2]  [64,2] ;  c = [b.a1, b.a2]  [1,2]
            v_ps = psumA.tile([D, 2], F32, tag="ps", name="v_ps")
            nc.tensor.matmul(v_ps, lhsT=W_sb, rhs=a_sb, start=True, stop=True)
            v_sb = small.tile([D, 2], F32)
            nc.scalar.copy(out=v_sb, in_=v_ps)

            c_ps = psumA.tile([1, 2], F32, tag="ps", name="c_ps")
            nc.tensor.matmul(c_ps, lhsT=b_sb, rhs=a_sb, start=True, stop=True)
            c_sb = small.tile([1, 2], F32)
            nc.scalar.copy(out=c_sb, in_=c_ps)

            # c12 = (c1 + c2) broadcast down 128 partitions
            cb_ps = psumA.tile([128, 2], F32, tag="ps", name="cb_ps")
            nc.tensor.matmul(cb_ps, lhsT=ones_row, rhs=c_sb, start=True, stop=True)
            c12 = small.tile([128, 1], F32)
            nc.vector.tensor_reduce(out=c12, in_=cb_ps, axis=AX_X, op=ADD)
            c12s = small.tile([128, 1], F32)
            nc.vector.tensor_scalar(
                out=c12s, in0=c12, scalar1=0.01, scalar2=None, op0=MUL
            )

            # ---------------- G2b for the own block ----------------
            # p2row = v2.T @ xT[:, 0:RB] ; G2b[j,i] = exp(-0.99*p2[i])
            G2b = persist.tile([128, RB], BF16)
            for h in range(2):
                p2r_ps = psumA.tile([1, 512], F32, tag="ps", name="p2r_ps")
                nc.tensor.matmul(
                    p2r_ps,
                    lhsT=v_sb[:, 1:2],
                    rhs=xT_sb[:, h * 512 : (h + 1) * 512],
                    start=True,
                    stop=True,
                )
                g2row = small.tile([1, 512], BF16, tag="g2row", name="g2row")
                nc.scalar.activation(out=g2row, in_=p2r_ps, func=EXP, scale=-0.99)
                gb_ps = psumA.tile([128, 512], F32, tag="ps", name="gb_ps")
                nc.tensor.matmul(
                    gb_ps, lhsT=ones_bf, rhs=g2row, start=True, stop=True
                )
                nc.scalar.copy(
                    out=G2b[:, h * 512 : (h + 1) * 512], in_=gb_ps
                )

            # ---------------- main stream ----------------
            # per 8-tile chunk: 8 tiny PE projections (s1 cols), 2 ACT exps
            # (E1/F1), then 8 DVE e-tiles + 16 accumulating PE matmuls
            E1c = small.tile([128, NT], F32)
            F1c = small.tile([128, NT], F32)
            acc0 = psumB.tile([128, 512], F32, tag="acc0", name="acc0")
            acc1 = psumB.tile([128, 512], F32, tag="acc1", name="acc1")
            accs = [acc0, acc1]
            for c in range(8):
                s1_ps = psumP.tile([128, 8], F32, tag="s1ps", name="s1_ps")
                for k in range(8):
                    t = 8 * c + k
                    nc.tensor.matmul(
                        s1_ps[:, k : k + 1],
                        lhsT=xT_sb[:, t * 128 : (t + 1) * 128],
                        rhs=v_sb[:, 0:1],
                        start=True,
                        stop=True,
                    )
                nc.scalar.activation(
                    out=E1c[:, 8 * c : 8 * (c + 1)],
                    in_=s1_ps,
                    func=EXP,
                    bias=c12,
                    scale=1.0,
                )
                nc.scalar.activation(
                    out=F1c[:, 8 * c : 8 * (c + 1)],
                    in_=s1_ps,
                    func=EXP,
                    bias=c12s,
                    scale=0.01,
                )
                for jt in range(8 * c, 8 * (c + 1)):
                    e_t = epool.tile([128, RB], BF16, tag="e", name="e_t")
                    # e[j,i] = max(G2b[j,i] * F1[j], E1[j])
                    nc.vector.tensor_scalar(
                        out=e_t,
                        in0=G2b,
                        scalar1=F1c[:, jt : jt + 1],
                        scalar2=E1c[:, jt : jt + 1],
                        op0=MUL,
                        op1=MAX,
                    )
                    for h in range(2):
                        nc.tensor.matmul(
                            accs[h],
                            lhsT=x_bf[:, jt, 0:128],
                            rhs=e_t[:, h * 512 : (h + 1) * 512],
                            start=(jt == 0),
                            stop=(jt == NT - 1),
                        )

            # ---------------- epilogue: normalize + store ----------------
            outT = small.tile([D + 1, RB], F32)
            for h in range(2):
                nc.scalar.copy(
                    out=outT[:, h * 512 : (h + 1) * 512],
                    in_=accs[h][0 : D + 1, :],
                )
            out_flat = small.tile([128, BT * D], F32)
            out_sb = out_flat.rearrange("p (t d) -> p t d", t=BT)
            for t in range(BT):
                tp2 = psumA.tile([128, D + 1], F32, tag="ps", name="tp2")
                nc.tensor.transpose(
                    tp2, outT[:, t * 128 : (t + 1) * 128], ident[: D + 1, : D + 1]
                )
                rcol = opool.tile([128, 1], F32, tag="rcol", name="rcol")
                nc.vector.reciprocal(rcol, tp2[:, D : D + 1])
                nc.vector.tensor_scalar(
                    out=out_sb[:, t, :],
                    in0=tp2[:, 0:D],
                    scalar1=rcol,
                    scalar2=None,
                    op0=MUL,
                )
            nc.sync.dma_start(out_d[:, :], out_flat)

    nc.finalize()
    return nc


def _execute(inputs: dict, trace: bool = False):
    x = np.ascontiguousarray(np.asarray(inputs["x"], dtype=np.float32))
    W = np.ascontiguousarray(np.asarray(inputs["W"], dtype=np.float32))
    b = np.ascontiguousarray(
        np.asarray(inputs["b"], dtype=np.float32).reshape(D, 1)
    )
    a = np.ascontiguousarray(
        np.asarray(inputs["a"], dtype=np.float32).reshape(2 * D, 1)
    )
    assert x.shape == (N, D) and W.shape == (D, D)

    nc = build_bass()
    pack0 = np.zeros((128, PKW), np.float32)
    pack0[0:D, 0:D] = W
    pack0[0:D, D] = b[:, 0]
    pack0[0:D, D + 1] = a[:D, 0]
    pack0[0:D, D + 2] = a[D:, 0]
    pack0[:, D + 3 : D + 131] = np.eye(128, dtype=np.float32)

    # host-side layout: d-major x.T and bf16 x_ext (p-major), with the
    # j-tile order rotated per core so the own i-block comes first
    xT_full = np.ascontiguousarray(x.T)  # [64, N]
    xe = np.concatenate(
        [x, np.ones((N, 1), np.float32), np.zeros((N, 127 - D), np.float32)],
        axis=1,
    )
    xbf_tiles = (
        xe.reshape(NT, 128, 128).transpose(1, 0, 2).astype(ml_dtypes.bfloat16)
    )  # [128, NT, 128]
    in_maps = []
    for c in range(NCORES):
        rot = np.roll(np.arange(NT), -c * BT)  # own tiles first
        xT_c = np.ascontiguousarray(
            xT_full.reshape(D, NT, 128)[:, rot, :].reshape(D, N)
        )
        xbf_c = np.ascontiguousarray(
            xbf_tiles[:, rot, :].reshape(128, NT * 128)
        )
        in_maps.append({"xT": xT_c, "xbf": xbf_c, "pack": pack0})
    res = run_bass_kernel_spmd(
        nc, in_maps, core_ids=list(range(NCORES)), trace=trace
    )
    # un-permute each core's output: (p, t*D+d) -> (t*128+p, d)
    outs = []
    for r in res.results:
        o = r["out"].reshape(128, BT, D).transpose(1, 0, 2).reshape(RB, D)
        outs.append(o)
    out = np.ascontiguousarray(np.concatenate(outs, axis=0))
    return out, res


def kernel(x, W, b, a):
    out, _ = _execute({"x": x, "W": W, "b": b, "a": a})
    return out


# revision 8
# speedup vs baseline: 1.2939x; 1.2939x over previous
"""GAT layer (N=8192, D=64) as a Bass/Tile kernel on 8 TRN2 NeuronCores.

Math (reference):
    h  = x @ W.T + b
    s1 = h @ a1 ; s2 = h @ a2                    # [N] each
    score[i,j] = s2[i] + s1[j]
    att = softmax_j(leaky_relu(score))
    out = att @ x

Reformulation:
    Fold the linear layer:  v = W.T @ [a1|a2], c_k = b.a_k
      p1 = x @ v1 ; p2 = x @ v2 ; sh1 = p1 + c1 + c2
    Softmax rows are shift invariant; subtracting p2[i] from row i gives
      e[j,i] = max( E1[j], F1[j] * G2[i] )
      E1 = exp(sh1), F1 = exp(0.01*sh1), G2[i] = exp(-0.99*p2[i])
    With per-j-row scalars E1[j], F1[j] and the broadcast tile
    G2b[j,i] = G2[i], each weight tile ([j part, i free]) is ONE DVE
    tensor_scalar op:  e = max(G2b * F1[j], E1[j]).
    A ones-column appended to x gives the softmax denominator in the same
    PSUM accumulation:  outT[0:64,i] += x_ext[j,:].T @ e[j,i].

Sharding: each core owns N/8 = 1024 query rows i (x replicated).  The
j-tile order is rotated per core on the host so the own i-block is
tiles 0..7 — fully SPMD, own-block data arrives first.

Engine schedule (per core):
  PE    : warm-up burst (HAM -> 2.4GHz before real work), v/c prologue,
          p2 columns (8 FD=1 matmuls), G2b broadcast, 128 accumulating
          main matmuls, epilogue transposes.
  GPSIMD: s1 = x*v1b mul + reduce per 8-tile chunk (f32).
  ACT   : exp table preload at t~0, E1/F1/G2 exps, epilogue scale.
  DVE   : the 64 fused mult+max e-tiles (critical stream), small copies.
  DMA   : two HWDGE rings (scalar: pack+xbkT, sync: x streams + out).
"""

import sys
import types

import ml_dtypes
import numpy as np

import concourse.bacc as bacc
import concourse.bass as bass
import concourse.mybir as mybir
import concourse.tile as tile
from concourse.bass_utils import run_bass_kernel_spmd


def _install_ntff_hook_shim():
    """The agent image's ``antenv`` lacks ``axon_hooks``; provide it so
    ``run_bass_kernel_spmd(trace=True)`` can capture NTFF profiles."""
    if "antenv.axon_hooks" in sys.modules:
        return
    try:
        from trn_agent_boot.trn_boot import _ntff_profile_via_ctypes

        hook = _ntff_profile_via_ctypes("/opt/axon/libaxon_pjrt.so")
        mod = types.ModuleType("antenv.axon_hooks")
        mod._hook = hook
        mod.get_axon_ntff_profile_hook = lambda: mod._hook
        mod.set_axon_ntff_profile_hook = lambda h: setattr(mod, "_hook", h)
        sys.modules["antenv.axon_hooks"] = mod
    except Exception:
        pass


_install_ntff_hook_shim()

N, D = 8192, 64
NCORES = 8
RB = N // NCORES          # rows (i) per core = 1024
NT = N // 128             # j tiles of 128 = 64
BT = RB // 128            # i tiles per core = 8
F32 = mybir.dt.float32
BF16 = mybir.dt.bfloat16
EXP = mybir.ActivationFunctionType.Exp
COPY = mybir.ActivationFunctionType.Copy
ADD = mybir.AluOpType.add
MUL = mybir.AluOpType.mult
MAX = mybir.AluOpType.max
AX_X = mybir.AxisListType.X
PKW = D + 131  # packed small-input width (W | b | a | ident)
NWARM = 6      # PE warm-up matmuls


def build_bass() -> bass.Bass:
    nc = bacc.Bacc(None)
    xp_d = nc.declare_dram_parameter("xp", [128, NT * D], F32, isOutput=False)
    xbf_d = nc.declare_dram_parameter(
        "xbf", [128, NT * 128], BF16, isOutput=False
    )
    xbkT_d = nc.declare_dram_parameter("xbkT", [D, RB], F32, isOutput=False)
    pk_d = nc.declare_dram_parameter("pack", [128, PKW], F32, isOutput=False)
    out_d = nc.declare_dram_parameter("out", [128, BT * D], F32, isOutput=True)

    with tile.TileContext(nc) as tc:
        with (
            tc.tile_pool(name="persist", bufs=1) as persist,
            tc.tile_pool(name="small", bufs=1) as small,
            tc.tile_pool(name="work", bufs=3) as work,
            tc.tile_pool(name="epool", bufs=12) as epool,
            tc.tile_pool(name="opool", bufs=2) as opool,
            tc.tile_pool(name="psumA", bufs=2, space="PSUM") as psumA,
            tc.tile_pool(name="psumB", bufs=1, space="PSUM") as psumB,
            tc.tile_pool(name="psumG", bufs=2, space="PSUM") as psumG,
            tc.tile_pool(name="psumR", bufs=2, space="PSUM") as psumR,
        ):
            # ---- t~0: constants, ACT exp-table preload, PE warm-up ----
            ones_row = small.tile([1, 128], F32)
            nc.vector.memset(ones_row, 1.0)
            ones_bf = small.tile([1, 512], BF16)
            nc.vector.memset(ones_bf, 1.0)
            dummy_e = small.tile([1, 1], F32)
            # first ACTIVATE triggers the ~2.7us exp table load; bury it
            # under the initial DMA wait
            nc.scalar.activation(out=dummy_e, in_=ones_row[0:1, 0:1], func=EXP)
            # PE warm-up: HAM un-throttles after ~3.4us of activity, so
            # burn the DMA wait with junk matmuls to start warm
            junk_ps = psumG.tile([128, 512], F32, tag="gb", name="junk_ps")
            for w in range(NWARM):
                nc.tensor.matmul(
                    junk_ps,
                    lhsT=ones_bf[0:1, 0:128],
                    rhs=ones_bf,
                    start=True,
                    stop=True,
                )

            # ---- small inputs (scalar HWDGE ring, concurrent with sync) ----
            pk = small.tile([128, PKW], F32)
            nc.scalar.dma_start(pk, pk_d[:, :])
            W_sb = pk[0:D, 0:D]
            b_sb = pk[0:D, D : D + 1]
            a_sb = pk[0:D, D + 1 : D + 3]
            ident = pk[:, D + 3 : D + 3 + 128]
            xbkT_sb = small.tile([D, RB], F32)
            nc.scalar.dma_start(xbkT_sb, xbkT_d[:, :])

            # ---- x streams (sync ring), interleaved by need-time ----
            x_flat = persist.tile([128, NT * D], F32)
            x_sb = x_flat.rearrange("p (t d) -> p t d", t=NT)
            xbf_flat = persist.tile([128, NT * 128], BF16)
            x_bf = xbf_flat.rearrange("p (t d) -> p t d", t=NT)
            CWB = 16 * 128

            def xp_dma(c):
                nc.sync.dma_start(
                    x_flat[:, 8 * c * D : 8 * (c + 1) * D],
                    xp_d[:, 8 * c * D : 8 * (c + 1) * D],
                )

            def xbf_dma(k):
                nc.sync.dma_start(
                    xbf_flat[:, k * CWB : (k + 1) * CWB],
                    xbf_d[:, k * CWB : (k + 1) * CWB],
                )

            xp_dma(0)
            xbf_dma(0)
            xp_dma(1)
            xp_dma(2)
            xbf_dma(1)
            xp_dma(3)
            xp_dma(4)
            xbf_dma(2)
            xp_dma(5)
            xp_dma(6)
            xbf_dma(3)
            xp_dma(7)

            # ---------------- tiny projections on PE ----------------
            v_ps = psumA.tile([D, 2], F32, tag="ps", name="v_ps")
            nc.tensor.matmul(v_ps, lhsT=W_sb, rhs=a_sb, start=True, stop=True)
            v_sb = small.tile([D, 2], F32)
            nc.vector.tensor_copy(v_sb, v_ps)

            c_ps = psumA.tile([1, 2], F32, tag="ps", name="c_ps")
            nc.tensor.matmul(c_ps, lhsT=b_sb, rhs=a_sb, start=True, stop=True)
            c_sb = small.tile([1, 2], F32)
            nc.vector.tensor_copy(c_sb, c_ps)

            # c12 = (c1 + c2) broadcast down 128 partitions
            cb_ps = psumA.tile([128, 2], F32, tag="ps", name="cb_ps")
            nc.tensor.matmul(cb_ps, lhsT=ones_row, rhs=c_sb, start=True, stop=True)
            c12 = small.tile([128, 1], F32)
            nc.vector.tensor_reduce(out=c12, in_=cb_ps, axis=AX_X, op=ADD)
            c12s = small.tile([128, 1], F32)
            nc.vector.tensor_scalar(
                out=c12s, in0=c12, scalar1=0.01, scalar2=None, op0=MUL
            )

            # v rows [2,64] via PE transpose, then v1 broadcast [128,64]
            vr_ps = psumA.tile([2, D], F32, tag="ps", name="vr_ps")
            nc.tensor.transpose(vr_ps, v_sb, ident[:D, :D])
            vrow = small.tile([2, D], F32)
            nc.vector.tensor_copy(vrow, vr_ps)
            v1b_ps = psumA.tile([128, D], F32, tag="ps", name="v1b_ps")
            nc.tensor.matmul(
                v1b_ps, lhsT=ones_row, rhs=vrow[0:1, :], start=True, stop=True
            )
            v1b = small.tile([128, D], F32)
            nc.vector.tensor_copy(v1b, v1b_ps)
            v1b_b = bass.AP(
                tensor=v1b.tensor,
                offset=v1b.offset,
                ap=[v1b.ap[0], [0, 8], v1b.ap[1]],
            )

            # ---------------- G2b: p2 cols -> row -> exp -> bcast ----------
            # p2c[p,t] = sum_d xbkT[d, t*128+p] * v2[d]   (8 FD=1 matmuls)
            p2c_ps = psumA.tile([128, BT], F32, tag="ps", name="p2c_ps")
            for t in range(BT):
                nc.tensor.matmul(
                    p2c_ps[:, t : t + 1],
                    lhsT=xbkT_sb[:, t * 128 : (t + 1) * 128],
                    rhs=v_sb[:, 1:2],
                    start=True,
                    stop=True,
                )
            p2c = small.tile([128, BT], F32)
            nc.vector.tensor_copy(p2c, p2c_ps)
            # column transposes into [1, 512] rows (base partition 0)
            g2row = small.tile([1, RB], BF16)
            G2b = persist.tile([128, RB], BF16)
            for h in range(2):
                p2r_ps = psumR.tile([1, 512], F32, tag="p2r", name="p2r_ps")
                for t in range(4):
                    nc.tensor.transpose(
                        p2r_ps[:, t * 128 : (t + 1) * 128],
                        p2c[:, 4 * h + t : 4 * h + t + 1],
                        ident[:128, :128],
                    )
                nc.scalar.activation(
                    out=g2row[:, h * 512 : (h + 1) * 512],
                    in_=p2r_ps,
                    func=EXP,
                    scale=-0.99,
                )
                gb_ps = psumG.tile([128, 512], F32, tag="gb", name="gb_ps")
                nc.tensor.matmul(
                    gb_ps,
                    lhsT=ones_bf[0:1, 0:128],
                    rhs=g2row[:, h * 512 : (h + 1) * 512],
                    start=True,
                    stop=True,
                )
                nc.vector.tensor_copy(G2b[:, h * 512 : (h + 1) * 512], gb_ps)

            # ---------------- s1 pipeline: gpsimd mul + DVE reduce ---------
            s1c = small.tile([128, NT], F32)
            E1c = small.tile([128, NT], F32)
            F1c = small.tile([128, NT], F32)

            def s1_chunk(c):
                tmp = work.tile([128, 8, D], F32, tag="tmp", name="tmp")
                nc.gpsimd.tensor_mul(
                    tmp, x_sb[:, 8 * c : 8 * (c + 1), :], v1b_b
                )
                nc.vector.tensor_reduce(
                    out=s1c[:, 8 * c : 8 * (c + 1)], in_=tmp, axis=AX_X, op=ADD
                )
                nc.scalar.activation(
                    out=E1c[:, 8 * c : 8 * (c + 1)],
                    in_=s1c[:, 8 * c : 8 * (c + 1)],
                    func=EXP,
                    bias=c12,
                    scale=1.0,
                )
                nc.scalar.activation(
                    out=F1c[:, 8 * c : 8 * (c + 1)],
                    in_=s1c[:, 8 * c : 8 * (c + 1)],
                    func=EXP,
                    bias=c12s,
                    scale=0.01,
                )

            # first two chunks resolve before the e-stream starts
            s1_chunk(0)
            s1_chunk(1)

            # ---------------- main stream: DVE e-tiles + PE matmuls --------
            acc0 = psumB.tile([128, 512], F32, tag="acc0", name="acc0")
            acc1 = psumB.tile([128, 512], F32, tag="acc1", name="acc1")
            accs = [acc0, acc1]
            for jt in range(NT):
                if jt % 8 == 0 and jt // 8 + 2 < 8:
                    s1_chunk(jt // 8 + 2)
                e_t = epool.tile([128, RB], BF16, tag="e", name="e_t")
                # e[j,i] = max(G2b[j,i] * F1[j], E1[j])
                nc.vector.tensor_scalar(
                    out=e_t,
                    in0=G2b,
                    scalar1=F1c[:, jt : jt + 1],
                    scalar2=E1c[:, jt : jt + 1],
                    op0=MUL,
                    op1=MAX,
                )
                for h in range(2):
                    nc.tensor.matmul(
                        accs[h],
                        lhsT=x_bf[:, jt, 0:128],
                        rhs=e_t[:, h * 512 : (h + 1) * 512],
                        start=(jt == 0),
                        stop=(jt == NT - 1),
                    )

            # ---------------- epilogue: normalize + store ----------------
            outT = small.tile([D + 1, RB], F32)
            for h in range(2):
                nc.scalar.copy(
                    out=outT[:, h * 512 : (h + 1) * 512],
                    in_=accs[h][0 : D + 1, :],
                )
            out_flat = small.tile([128, BT * D], F32)
            out_sb = out_flat.rearrange("p (t d) -> p t d", t=BT)
            for t in range(BT):
                tp2 = psumA.tile([128, D + 1], F32, tag="ps", name="tp2")
                nc.tensor.transpose(
                    tp2, outT[:, t * 128 : (t + 1) * 128], ident[: D + 1, : D + 1]
                )
                rcol = opool.tile([128, 1], F32, tag="rcol", name="rcol")
                nc.vector.reciprocal(rcol, tp2[:, D : D + 1])
                nc.vector.tensor_scalar(
                    out=out_sb[:, t, :],
                    in0=tp2[:, 0:D],
                    scalar1=rcol,
                    scalar2=None,
                    op0=MUL,
                )
                if t == 3:
                    nc.sync.dma_start(
                        out_d[:, 0 : 4 * D], out_flat[:, 0 : 4 * D]
                    )
            nc.sync.dma_start(out_d[:, 4 * D :], out_flat[:, 4 * D :])

    nc.finalize()
    return nc


def _execute(inputs: dict, trace: bool = False):
    x = np.ascontiguousarray(np.asarray(inputs["x"], dtype=np.float32))
    W = np.ascontiguousarray(np.asarray(inputs["W"], dtype=np.float32))
    b = np.ascontiguousarray(
        np.asarray(inputs["b"], dtype=np.float32).reshape(D, 1)
    )
    a = np.ascontiguousarray(
        np.asarray(inputs["a"], dtype=np.float32).reshape(2 * D, 1)
    )
    assert x.shape == (N, D) and W.shape == (D, D)

    nc = build_bass()
    pack0 = np.zeros((128, PKW), np.float32)
    pack0[0:D, 0:D] = W
    pack0[0:D, D] = b[:, 0]
    pack0[0:D, D + 1] = a[:D, 0]
    pack0[0:D, D + 2] = a[D:, 0]
    pack0[:, D + 3 : D + 131] = np.eye(128, dtype=np.float32)

    # host-side layout only: partition-major f32 x, bf16 x_ext, d-major
    # own-block x.T; j-tile order rotated per core (own block first)
    xp_tiles = x.reshape(NT, 128, D).transpose(1, 0, 2)  # [128, NT, D]
    xe = np.concatenate(
        [x, np.ones((N, 1), np.float32), np.zeros((N, 127 - D), np.float32)],
        axis=1,
    )
    xbf_tiles = (
        xe.reshape(NT, 128, 128).transpose(1, 0, 2).astype(ml_dtypes.bfloat16)
    )  # [128, NT, 128]
    in_maps = []
    for c in range(NCORES):
        rot = np.roll(np.arange(NT), -c * BT)  # own tiles first
        xp_c = np.ascontiguousarray(xp_tiles[:, rot, :].reshape(128, NT * D))
        xbf_c = np.ascontiguousarray(
            xbf_tiles[:, rot, :].reshape(128, NT * 128)
        )
        xbkT = np.ascontiguousarray(x[c * RB : (c + 1) * RB].T)
        in_maps.append(
            {"xp": xp_c, "xbf": xbf_c, "xbkT": xbkT, "pack": pack0}
        )
    res = run_bass_kernel_spmd(
        nc, in_maps, core_ids=list(range(NCORES)), trace=trace
    )
    # un-permute each core's output: (p, t*D+d) -> (t*128+p, d)
    outs = []
    for r in res.results:
        o = r["out"].reshape(128, BT, D).transpose(1, 0, 2).reshape(RB, D)
        outs.append(o)
    out = np.ascontiguousarray(np.concatenate(outs, axis=0))
    return out, res


def kernel(x, W, b, a):
    out, _ = _execute({"x": x, "W": W, "b": b, "a": a})
    return out
